# revision 11
# baseline (speedup 1.0000x reference)
"""Trainium2 Bass kernel for nn_Model2_8340826488964 (dense_mlp, recurrent+syncBN).

Model per timestep t (T=512, B=2048, NH=1024, NV=31):
    h = relu((h_prev + emb[x_t]) @ W_hh.T + b_hh)
    BN over batch (training stats), out_t = BN(h) @ W_ho.T + b_ho

Strategy: data-parallel over batch (256 rows/core on 8 cores).
 - Layout: features on partitions (8 f-tiles x 128), batch on free axis (256).
 - Recurrence in bf16.
 - emb-add folded into the matmul via e2 = emb @ W_hh.T and a host-precomputed
   one-hot streamed from HBM. The 8 K=31 e2 matmuls per step are packed 4-wide
   into 32-row PE bands (tile_position row tiling): 2 spans instead of 8
   serial matmuls, issued FIRST each step (no h dependency).
 - BN stats: ACT relu pass emits per-feature sums (accum_out); DVE STT emits
   sum-of-squares. Stats for G steps batched into ONE AllReduce (overlapped).
 - Output projection in [v, b] orientation with the BN scale folded into W_ho
   (ws = a_t * W_ho per step, tiny DVE ops) and the BN offset folded into a
   per-group constant chain (psc). mm2 for 4 steps runs concurrently in 4
   32-partition column bands (tile_position col tiling): 8 N=256 spans per
   4 steps. Bias applied by ACT during PSUM->SBUF copy; DMA out per group.
"""
import sys, os
sys.path.insert(0, "/opt/trn_rl_repo")
import numpy as np
import ml_dtypes

from concourse import bass, bacc, tile, bass_utils
from concourse import mybir
from concourse.bass_interp import get_hw_module

BF16 = ml_dtypes.bfloat16

N_CORES = 8
B, T_FULL, NH, NV = 2048, 512, 1024, 31
BC = B // N_CORES            # 256 batch rows per core
NF = NH // 128               # 8 feature tiles
BN_EPS = 1e-5

G = 8                        # steps per stats-allreduce group
D = 14                       # h ring depth (needs >= G+6 for quad mm2)

F32 = mybir.dt.float32
BF = mybir.dt.bfloat16
AF = mybir.ActivationFunctionType
OP = mybir.AluOpType


def build(T: int, g: int = G, d: int = D, no_cc: bool = False,
          pack_e2: bool = True, quad_mm2: bool = True):
    assert T % g == 0 and g % 4 == 0
    nc = bacc.Bacc("TRN2", target_bir_lowering=False, debug=False,
                   enable_asserts=False, num_devices=N_CORES)

    whh_d = nc.dram_tensor("whh", [128, 64 * 128], BF, kind="ExternalInput").ap()
    e2r_d = nc.dram_tensor("e2r", [128, NH], BF, kind="ExternalInput").ap()
    whot_d = nc.dram_tensor("whot", [128, NF * NV], BF, kind="ExternalInput").ap()
    whor_d = nc.dram_tensor("whor", [128, NF * 128], BF, kind="ExternalInput").ap()
    bhh_d = nc.dram_tensor("bhh", [128, NF], F32, kind="ExternalInput").ap()
    gamr_d = nc.dram_tensor("gamr", [128, NF * g], F32, kind="ExternalInput").ap()
    betr_d = nc.dram_tensor("betr", [128, NF * g], F32, kind="ExternalInput").ap()
    bhor_d = nc.dram_tensor("bhor", [128, 1], F32, kind="ExternalInput").ap()
    oneh_d = nc.dram_tensor("oneh", [128, T * BC], BF, kind="ExternalInput").ap()
    out_d = nc.dram_tensor("out_shard", [NV, T, BC], F32, kind="ExternalOutput").ap()

    n_groups = T // g
    inv_n = 1.0 / float(B)

    with tile.TileContext(nc) as tc:
        with tc.tile_pool(name="const", bufs=1) as cpool, \
             tc.tile_pool(name="hring", bufs=d) as hpool, \
             tc.tile_pool(name="ws", bufs=8) as wspool, \
             tc.tile_pool(name="oneh", bufs=3) as opool, \
             tc.tile_pool(name="stats", bufs=2) as spool, \
             tc.tile_pool(name="fin", bufs=2) as fpool, \
             tc.tile_pool(name="acc", bufs=2) as accpool, \
             tc.tile_pool(name="scratch", bufs=1) as scpool, \
             tc.tile_pool(name="ps1", bufs=5, space="PSUM") as ps1pool, \
             tc.tile_pool(name="psv", bufs=2, space="PSUM") as psvpool, \
             tc.tile_pool(name="psc", bufs=1, space="PSUM") as pscpool, \
             tc.tile_pool(name="dram", bufs=4, space="DRAM") as dpool:

            # ---- load constants ----
            whh = cpool.tile([128, 64 * 128], BF, tag="whh", name="whh")
            e2r = cpool.tile([128, NH], BF, tag="e2r", name="e2r")
            whot = cpool.tile([128, NF * NV], BF, tag="whot", name="whot")
            whor = cpool.tile([128, NF * 128], BF, tag="whor", name="whor")
            bhh = cpool.tile([128, NF], F32, tag="bhh", name="bhh")
            gamr = cpool.tile([128, NF * g], F32, tag="gamr", name="gamr")
            betr = cpool.tile([128, NF * g], F32, tag="betr", name="betr")
            bhor = cpool.tile([128, 1], F32, tag="bhor", name="bhor")
            nc.sync.dma_start(whh[:], whh_d[:])
            nc.sync.dma_start(e2r[:], e2r_d[:])
            nc.sync.dma_start(whot[:], whot_d[:])
            nc.sync.dma_start(whor[:], whor_d[:])
            nc.sync.dma_start(bhh[:], bhh_d[:])
            nc.sync.dma_start(gamr[:], gamr_d[:])
            nc.sync.dma_start(betr[:], betr_d[:])
            nc.sync.dma_start(bhor[:], bhor_d[:])

            sq_scr = scpool.tile([128, 256], BF, tag="sqscr", name="sqscr")
            z1 = scpool.tile([1, 384], BF, tag="z1", name="z1")
            nc.vector.memset(z1[:], 0.0)

            h_tiles = {}        # s -> h tile [128, NF*256] bf16
            oneh_tiles = {}     # group -> [128, g*256] bf16 (4x replicated rows)
            stats_loc = {}      # group -> [128, 16*g] f32 (sums | sumsqs)
            stats_glb = {}      # group -> [128, 16*g] f32
            ag_cg = {}          # group -> (a_g f32 [128,8g], c_gb bf16 [128,8g])
            bias_rep = {}       # group -> [128, g] f32 (banded bias columns)
            acc_tiles = {}      # group -> [128, (g//4)*256] f32 banded output
            ws_tiles = {}       # t -> ws tile [128, NF*31] bf16

            # prefetch onehot for groups 0,1
            for gg in range(min(2, n_groups)):
                ot = opool.tile([128, g * BC], BF, tag="oneh", name="oneh")
                nc.sync.dma_start(ot[:], oneh_d[:, gg * g * BC:(gg + 1) * g * BC])
                oneh_tiles[gg] = ot

            for s in range(T + g + 1):
                # ======== forward recurrence step s ========
                if s < T:
                    u, gg = s % g, s // g
                    if u == 0:
                        stats_loc[gg] = spool.tile([128, 16 * g], F32, tag="sloc", name="sloc")
                        if gg + 2 < n_groups:
                            ot = opool.tile([128, g * BC], BF, tag="oneh", name="oneh")
                            nc.sync.dma_start(
                                ot[:], oneh_d[:, (gg + 2) * g * BC:(gg + 3) * g * BC])
                            oneh_tiles[gg + 2] = ot
                    sloc = stats_loc[gg]
                    h_t = hpool.tile([128, NF * 256], BF, tag="h", name="h")
                    h_tiles[s] = h_t
                    h_prev = h_tiles.get(s - 1)
                    oneh_g = oneh_tiles[gg]

                    # 4 single-bank psum tiles for this step (fi%4 -> bank)
                    bk = [ps1pool.tile([128, 512], F32, tag="ps1", name="ps1")
                          for _ in range(4)]

                    # -- main W_hh chains; e2 closes each quarter's group.
                    #    NOTE: start=True clears has_written for the WHOLE
                    #    bank, so each bank gets exactly one start per half
                    #    (the chain's ki==0); e2 accumulates (start=False)
                    #    and stops. --
                    for half in range(2):
                        for r in range(4):
                            fi = half * 4 + r
                            pslice = bk[r][:, half * 256:(half + 1) * 256]
                            if s > 0:
                              with nc.named_scope("chain"):
                                for ki in range(NF):
                                    # only half 0 opens the bank: start=True
                                    # clears has_written for the WHOLE bank
                                    # (both halves' columns), so half 1 can
                                    # overwrite-on-cleared-bits with
                                    # start=False and avoid a WAR stall
                                    # against ACT reading half 0's columns.
                                    nc.tensor.matmul(
                                        pslice,
                                        whh[:, (ki * NF + fi) * 128:(ki * NF + fi + 1) * 128],
                                        h_prev[:, ki * 256:(ki + 1) * 256],
                                        start=(ki == 0 and half == 0), stop=False,
                                        skip_group_check=True)
                            if not pack_e2:
                                nc.tensor.matmul(
                                    pslice,
                                    e2r[0:NV, fi * 128:(fi + 1) * 128],
                                    oneh_g[0:NV, u * BC:(u + 1) * BC],
                                    start=(s == 0 and half == 0), stop=True,
                                    skip_group_check=True)
                        if pack_e2:
                          with nc.named_scope("e2"):
                            for r in range(4):
                                fi = half * 4 + r
                                nc.tensor.matmul(
                                    bk[r][:, half * 256:(half + 1) * 256],
                                    e2r[32 * r:32 * r + NV, fi * 128:(fi + 1) * 128],
                                    oneh_g[32 * r:32 * r + NV, u * BC:(u + 1) * BC],
                                    start=(s == 0 and half == 0), stop=True,
                                    tile_position=(32 * r, 0),
                                    skip_group_check=True)
                        for r in range(4):
                          with nc.named_scope("relu"):
                            fi = half * 4 + r
                            pslice = bk[r][:, half * 256:(half + 1) * 256]
                            # ACT: h = relu(psum + b), accumulate per-feature sum
                            nc.scalar.activation(
                                h_t[:, fi * 256:(fi + 1) * 256], pslice,
                                AF.Relu, bias=bhh[:, fi:fi + 1], scale=1.0,
                                accum_out=sloc[:, fi * g + u:fi * g + u + 1])
                            nc.vector.scalar_tensor_tensor(
                                sq_scr[:], h_t[:, fi * 256:(fi + 1) * 256], 1.0,
                                h_t[:, fi * 256:(fi + 1) * 256],
                                op0=OP.mult, op1=OP.mult,
                                accum_out=sloc[:, 8 * g + fi * g + u:8 * g + fi * g + u + 1])

                    if u == g - 1:
                        # group complete: allreduce the stats
                        cin = dpool.tile([128, 16 * g], F32, tag="ccin", name="ccin")
                        cout = dpool.tile([128, 16 * g], F32, tag="ccout", name="ccout",
                                          addr_space="Shared")
                        nc.gpsimd.dma_start(cin[:], sloc[:])
                        if no_cc:
                            nc.gpsimd.dma_start(cout[:], cin[:])
                        else:
                            nc.gpsimd.collective_compute(
                                "AllReduce", OP.add, ins=[cin[:]], outs=[cout[:]],
                                replica_groups=[list(range(N_CORES))])
                        sg = spool.tile([128, 16 * g], F32, tag="sglb", name="sglb")
                        nc.gpsimd.dma_start(sg[:], cout[:])
                        stats_glb[gg] = sg

                # ======== delayed BN + output path for step t = s-g-1 ========
                t = s - g - 1
                if 0 <= t < T:
                    ut, gt = t % g, t // g
                    if ut == 0:
                        # finalize group stats: a = gamma*rsqrt(var+eps),
                        # c = beta - mean*a  (layout col = fi*g + u)
                        sg = stats_glb[gt]
                        mean = fpool.tile([128, 8 * g], F32, tag="mean", name="mean")
                        ex2 = fpool.tile([128, 8 * g], F32, tag="ex2", name="ex2")
                        vep = fpool.tile([128, 8 * g], F32, tag="vep", name="vep")
                        rcp = fpool.tile([128, 8 * g], F32, tag="rcp", name="rcp")
                        rsq = fpool.tile([128, 8 * g], F32, tag="rsq", name="rsq")
                        a_g = fpool.tile([128, 8 * g], F32, tag="a_g", name="a_g")
                        c_g = fpool.tile([128, 8 * g], F32, tag="c_g", name="c_g")
                        c_gb = fpool.tile([128, 8 * g], BF, tag="c_gb", name="c_gb")
                        m2 = fpool.tile([128, 8 * g], F32, tag="m2", name="m2")
                        nc.vector.tensor_scalar(mean[:], sg[:, 0:8 * g], inv_n, None, OP.mult)
                        nc.vector.tensor_scalar(ex2[:], sg[:, 8 * g:16 * g], inv_n, None, OP.mult)
                        nc.vector.tensor_tensor(m2[:], mean[:], mean[:], OP.mult)
                        nc.vector.scalar_tensor_tensor(
                            vep[:], ex2[:], BN_EPS, m2[:], op0=OP.add, op1=OP.subtract)
                        nc.vector.reciprocal(rcp[:], vep[:])
                        nc.scalar.activation(rsq[:], rcp[:], AF.Sqrt)
                        nc.vector.tensor_tensor(a_g[:], rsq[:], gamr[:], OP.mult)
                        nc.vector.tensor_tensor(c_g[:], mean[:], a_g[:], OP.mult)
                        nc.vector.tensor_tensor(c_g[:], betr[:], c_g[:], OP.subtract)
                        nc.vector.tensor_copy(c_gb[:], c_g[:])
                        ag_cg[gt] = (a_g, c_gb)

                        # psc: banded ct[v,u] = sum_f c[f] W_ho[v,f] via whor
                        # (lhsT free = 4 bands of 32), then bias = ct + b_ho
                        psc = pscpool.tile([128, g], F32, tag="psc", name="psc")
                        sc_psc = nc.named_scope("psc"); sc_psc.__enter__()
                        for fi in range(NF):
                            nc.tensor.matmul(
                                psc[:, 0:g],
                                whor[:, fi * 128:(fi + 1) * 128],
                                c_gb[:, fi * g:(fi + 1) * g],
                                start=(fi == 0), stop=(fi == NF - 1))
                        sc_psc.__exit__(None, None, None)
                        br = fpool.tile([128, g], F32, tag="br", name="br")
                        nc.vector.tensor_scalar(br[:], psc[:, 0:g], bhor[:, 0:1],
                                                None, OP.add)
                        bias_rep[gt] = br
                        acc_tiles[gt] = accpool.tile([128, (g // 4) * 256], F32,
                                                     tag="acc", name="acc")

                    a_g, c_gb = ag_cg[gt]
                    # ws for step t: per f-tile, whot scaled by a (per-partition)
                    ws = wspool.tile([128, NF * NV], BF, tag="ws", name="ws")
                    ws_tiles[t] = ws
                    for fi in range(NF):
                        nc.vector.tensor_scalar(
                            ws[:, fi * NV:(fi + 1) * NV],
                            whot[:, fi * NV:(fi + 1) * NV],
                            a_g[:, fi * g + ut:fi * g + ut + 1], None, OP.mult)

                    if quad_mm2:
                        if t % 4 == 3:
                            # mm2 for quad [t-3 .. t]: 4 col bands concurrently
                            tau = t - 3
                            m = ut // 4   # block within group (0 or 1)
                            psv = psvpool.tile([128, 256], F32, tag="psv", name="psv")
                            tc2 = nc.named_scope("quad"); tc2.__enter__()
                            # open the bank once (start=True clears the whole
                            # bank's has_written): K=1 zero matmul over all
                            # bands, then every band MM accumulates.
                            nc.tensor.matmul(
                                psv[:, :], z1[0:1, 0:128], z1[0:1, 128:384],
                                start=True, stop=False, skip_group_check=True)
                            for k in range(NF):
                                for j in range(4):
                                    tt = tau + j
                                    nc.tensor.matmul(
                                        psv[32 * j:32 * j + NV, :],
                                        ws_tiles[tt][:, k * NV:(k + 1) * NV],
                                        h_tiles[tt][:, k * 256:(k + 1) * 256],
                                        start=False, stop=(k == NF - 1),
                                        tile_position=(0, 32 * j),
                                        skip_group_check=True)
                            acc = acc_tiles[gt]
                            br = bias_rep[gt]
                            for j in range(4):
                                uu = (tau + j) % g
                                nc.scalar.activation(
                                    acc[32 * j:32 * j + NV, m * 256:(m + 1) * 256],
                                    psv[32 * j:32 * j + NV, :],
                                    AF.Identity, bias=br[32 * j:32 * j + NV, uu:uu + 1],
                                    scale=1.0)
                            tc2.__exit__(None, None, None)
                            for j in range(4):
                                del ws_tiles[tau + j]
                                h_tiles.pop(tau + j)
                    else:
                        # serial mm2 (fallback): 8 MMs into band 0
                        psv = psvpool.tile([128, 256], F32, tag="psv", name="psv")
                        h_old = h_tiles[t]
                        for k in range(NF):
                            nc.tensor.matmul(
                                psv[0:NV, :],
                                ws[:, k * NV:(k + 1) * NV],
                                h_old[:, k * 256:(k + 1) * 256],
                                start=(k == 0), stop=(k == NF - 1))
                        acc = acc_tiles[gt]
                        br = bias_rep[gt]
                        nc.scalar.activation(
                            acc[0:NV, 0:256], psv[0:NV, :],
                            AF.Identity, bias=br[0:NV, ut:ut + 1], scale=1.0)
                        nc.sync.dma_start(
                            out_d[:, t:t + 1, :], acc[0:NV, 0:256])
                        del ws_tiles[t]
                        h_tiles.pop(t)

                    if quad_mm2 and ut == g - 1:
                        # group complete: DMA banded acc -> out_d[NV, T, BC]
                        acc = acc_tiles[gt]
                        nb = g // 4
                        for j in range(4):
                            for mm in range(nb):
                                tt = gt * g + mm * 4 + j
                                nc.sync.dma_start(
                                    out_d[:, tt:tt + 1, :],
                                    acc[32 * j:32 * j + NV, mm * 256:(mm + 1) * 256])
                        del acc_tiles[gt], stats_glb[gt], stats_loc[gt]
                        del ag_cg[gt], bias_rep[gt]
                        if gt in oneh_tiles:
                            del oneh_tiles[gt]

    nc.compile()
    nc.m = get_hw_module(nc.m)
    return nc


def prep_inputs(x, emb, W_hh, b_hh, W_ho, b_ho, gamma, beta, T):
    """Host-side packing. Returns in_maps (list of per-core dicts)."""
    x = np.asarray(x)
    emb = np.asarray(emb, np.float32)
    W_hh = np.asarray(W_hh, np.float32)
    b_hh = np.asarray(b_hh, np.float32)
    W_ho = np.asarray(W_ho, np.float32)
    b_ho = np.asarray(b_ho, np.float32)
    gamma = np.asarray(gamma, np.float32)
    beta = np.asarray(beta, np.float32)
    g = G

    WT = np.ascontiguousarray(W_hh.T)                      # [k, f]
    whh = WT.reshape(NF, 128, NF, 128).transpose(1, 0, 2, 3)  # [k_l, ki, fi, f_l]
    whh = np.ascontiguousarray(whh.reshape(128, 64 * 128)).astype(BF16)

    e2 = (emb @ W_hh.T).astype(np.float32)                 # [31, 1024]
    e2r = np.zeros((128, NH), dtype=BF16)
    for r in range(4):
        e2r[32 * r:32 * r + NV, :] = e2.astype(BF16)

    whot = np.ascontiguousarray(
        W_ho.T.reshape(NF, 128, NV).transpose(1, 0, 2).reshape(128, NF * NV)).astype(BF16)

    # whor: [128, NF*128], for k-tile fi: cols fi*128 + 32j + v = W_ho[v, fi*128+p]
    whor = np.zeros((128, NF * 128), dtype=BF16)
    wt = W_ho.T.reshape(NF, 128, NV)                       # [fi, f_l, v]
    for fi in range(NF):
        for j in range(4):
            whor[:, fi * 128 + 32 * j: fi * 128 + 32 * j + NV] = wt[fi].astype(BF16)

    bhh2 = np.ascontiguousarray(b_hh.reshape(NF, 128).T)   # [128, 8]
    gam2 = np.ascontiguousarray(gamma.reshape(NF, 128).T)  # [128, 8]
    bet2 = np.ascontiguousarray(beta.reshape(NF, 128).T)
    gamr = np.repeat(gam2, g, axis=1)                      # [128, 8g] col=fi*g+u
    betr = np.repeat(bet2, g, axis=1)

    bhor = np.zeros((128, 1), dtype=np.float32)
    for j in range(4):
        bhor[32 * j:32 * j + NV, 0] = b_ho

    common = dict(whh=whh, e2r=e2r, whot=whot, whor=whor, bhh=bhh2,
                  gamr=gamr, betr=betr, bhor=bhor)
    in_maps = []
    t_idx = np.arange(T)[:, None]
    b_idx = np.arange(BC)[None, :]
    for c in range(N_CORES):
        xc = x[c * BC:(c + 1) * BC, :T].T                  # [T, 256]
        oh = np.zeros((128, T, BC), dtype=BF16)
        for r in range(4):
            oh[32 * r + xc, t_idx, b_idx] = 1
        m = dict(common)
        m["oneh"] = oh.reshape(128, T * BC)
        in_maps.append(m)
    return in_maps


_CACHE = {}

def _get_built(T):
    if T not in _CACHE:
        _CACHE[T] = build(T)
    return _CACHE[T]


def run(inputs, T=T_FULL, trace=False):
    nc = _get_built(T)
    in_maps = prep_inputs(inputs["x"], inputs["emb"], inputs["W_hh"], inputs["b_hh"],
                          inputs["W_ho"], inputs["b_ho"], inputs["gamma"],
                          inputs["beta"], T)
    res = bass_utils.run_bass_kernel_spmd(
        nc, in_maps, core_ids=list(range(N_CORES)), trace=trace)
    out = np.concatenate(
        [np.ascontiguousarray(res.results[c]["out_shard"].transpose(2, 1, 0))
         for c in range(N_CORES)], axis=0)
    return out, res


def kernel(**inputs) -> np.ndarray:
    out, _ = run(inputs, T=T_FULL, trace=False)
    return out
